# revision 30
# baseline (speedup 1.0000x reference)
"""Gaussian RBF kernel-mean loss on 8 Trainium2 NeuronCores.

Computes mean(exp(-||x_i - y_j||^2 / 2)) over all (i, j) pairs for
x, y of shape [8192, 256] fp32.

Math used on device (per core, rows of x sharded 1024/core):
    exp(-d2/2) = exp(x.y - 0.5||x||^2 - 0.5||y||^2)
Features ship as s*x, s*y in fp8 e4m3 (host picks s; s=1 fast path for
unit-scale inputs). The column term -0.5||s y_n||^2 rides in the
contraction as two error-compensated bf16 rows (c1 = bf16(v),
c2 = bf16(v - c1)) multiplied by constant-1 rows on the x side:
    psum = s^2 x.y + c1 + c2       # PE, fp32 accumulate, mixed-dtype group
    E    = exp(psum/s^2 + bias_m)  # ACT: scale=1/s^2 (xb last col),
                                   #      bias = per-partition -0.5||x_m||^2
    stats[:, t] = sum_n E          # ACT accum_out, fp32
    out[p] = sum_t stats[p, t]     # DVE tensor_reduce
The host adds the 8 * 128 partials and divides by N*M.

Distribution: a call's wall-clock is dominated by shipping bytes over
the tunneled PJRT link (the device kernel itself is ~100us), so each
core receives only its own x shard and y shard (~0.5MB fp8 per core);
the full y^T is assembled ON DEVICE with a DRAM AllGather across the 8
cores. Total host->device traffic ~4.4MB vs 54.6MB for the replicated
bf16 layout of the original version.

Execution: the first call goes through bass_utils.run_bass_kernel_spmd
(which compiles the NEFF). Subsequent calls reuse a process-cached
jax.jit(shard_map) wrapper built on the same bass2jax primitives, which
skips the per-call retrace + walrus recompile that run_bass_kernel_spmd
pays (it constructs a fresh jit closure per invocation).

Toolchain constraint: this walrus build accepts at most ONE sync wait
per compute/DMA instruction. The kernel keeps a strict PE -> ACT
pipeline; slot-recycle and DMA-arrival waits are absorbed by tiny
same-engine "observer" ops (LDWEIGHTS on PE, a scalar warmup on ACT),
and any instruction still over budget gets its excess waits split into
single-wait same-engine drains (_strip_self_waits/_rebalance_waits).
"""

import numpy as np
import ml_dtypes

N = 8192          # rows of x
M = 8192          # rows of y
K = 256           # feature dim
NCORES = 8
MPC = N // NCORES        # 1024 rows of x (and y) per core
P = 128                  # partitions
KO = K // P              # 2 k-chunks
KA = K + 2               # y shard rows incl c1, c2 compensation rows
MB = MPC // P            # 8 m-blocks per core
NG_W = 2048              # columns per psum tile (4 banks)
NG = M // NG_W           # 4 n-groups
NS_W = 512               # matmul free width (1 psum bank)
NS = NG_W // NS_W        # 4
NTILES = MB * NG         # 32 output tiles per core

_cached = {}
_last_in_maps = None


def _build(fix_waits=True):
    import concourse.bass as bass
    import concourse.tile as tile
    import concourse.mybir as mybir
    from contextlib import ExitStack

    fp32 = mybir.dt.float32
    bf16 = mybir.dt.bfloat16
    f8 = mybir.dt.float8e4

    nc = bass.Bass(trn_type="TRN2")
    # xy packs this core's x.T shard (rows 0..K-1) and y.T shard
    # (rows K..2K-1) into ONE input: a single device_put per call ships
    # both, halving the ~30-40ms fixed cost each put carries on the
    # tunneled PJRT link.
    xy = nc.dram_tensor("xy", [2 * K, MPC], f8, kind="ExternalInput")
    ya = nc.dram_tensor("ya", [2, MPC], bf16, kind="ExternalInput")
    xb = nc.dram_tensor("xb", [P, MB + 1], fp32, kind="ExternalInput")
    stats = nc.dram_tensor("stats", [P, 1], fp32, kind="ExternalOutput")

    with ExitStack() as ctx:
        tc = ctx.enter_context(tile.TileContext(nc))
        singles = ctx.enter_context(tc.tile_pool(name="singles", bufs=1))
        dram = ctx.enter_context(tc.tile_pool(name="dram", bufs=1, space="DRAM"))
        psum_pool = ctx.enter_context(
            tc.tile_pool(name="psum", bufs=2, space="PSUM")
        )
        e_pool = ctx.enter_context(tc.tile_pool(name="e", bufs=4))

        in_f = dram.tile([K, MPC], f8)
        in_a = dram.tile([2, MPC], bf16)
        ytg = dram.tile([NCORES, K, MPC], f8)
        ytga = dram.tile([NCORES, 2, MPC], bf16)

        xt_sb = singles.tile([P, KO, MPC], f8)
        ytg_sb = singles.tile([P, KO, M], f8)
        yaug_sb = singles.tile([2, M], bf16)
        ones_sb = singles.tile([2, P], bf16)
        xb_sb = singles.tile([P, MB + 1], fp32)
        st_sb = singles.tile([P, NTILES], fp32)
        red_sb = singles.tile([P, 1], fp32)
        warm = singles.tile([P, 1], fp32)

        # y shard -> DRAM bounce -> AllGather to full y^T (+aug rows)
        nc.gpsimd.dma_start(out=in_f, in_=xy[K : 2 * K, :])
        nc.gpsimd.dma_start(out=in_a, in_=ya.ap())
        nc.gpsimd.collective_compute(
            "AllGather",
            mybir.AluOpType.bypass,
            replica_groups=[list(range(NCORES))],
            ins=[in_f.opt()],
            outs=[ytg.opt()],
        )
        nc.gpsimd.collective_compute(
            "AllGather",
            mybir.AluOpType.bypass,
            replica_groups=[list(range(NCORES))],
            ins=[in_a.opt()],
            outs=[ytga.opt()],
        )

        for ko in range(KO):
            nc.sync.dma_start(
                out=xt_sb[:, ko], in_=xy[ko * P : (ko + 1) * P, :]
            )
        nc.sync.dma_start(out=xb_sb, in_=xb.ap())
        nc.vector.memset(ones_sb, 1.0)
        # PE observer for the xt DMA queue (no PSUM write -> no bank WAW)
        nc.tensor.ldweights(weights=xt_sb[:, 0, 0:P])
        # ACT warmup: loads the exp table set AND observes the xb DMA queue,
        # so no later Exp carries the table-load's extra sync wait.
        nc.scalar.activation(
            out=warm, in_=xb_sb[:, 0:1], func=mybir.ActivationFunctionType.Exp
        )
        # gathered y columns: per source core j, feature chunks + aug rows
        for j in range(NCORES):
            cs = slice(j * MPC, (j + 1) * MPC)
            for ko in range(KO):
                nc.sync.dma_start(
                    out=ytg_sb[:, ko, cs],
                    in_=ytg[j, ko * P : (ko + 1) * P, :],
                )
            nc.sync.dma_start(out=yaug_sb[:, cs], in_=ytga[j])

        e_list = []
        t = 0
        for mb in range(MB):
            ms = slice(mb * P, (mb + 1) * P)
            for ng in range(NG):
                if mb == 0:
                    # PE observer: absorb the ytg shard DMA-arrival waits
                    j0 = ng * (NG_W // MPC)
                    c0 = j0 * MPC
                    nc.tensor.ldweights(weights=ytg_sb[:, 0, c0 : c0 + P])
                if t >= 2:
                    # PE observer: absorb the psum-slot-recycle wait
                    # (ACT finished exp of tile t-2).
                    nc.tensor.ldweights(weights=e_list[t - 2][:, 0:P])
                psum = psum_pool.tile([P, NG_W], fp32)
                for ns in range(NS):
                    c0 = ng * NG_W + ns * NS_W
                    out_sl = psum[:, ns * NS_W : (ns + 1) * NS_W]
                    nc.tensor.matmul(
                        out_sl,
                        xt_sb[:, 0, ms],
                        ytg_sb[:, 0, c0 : c0 + NS_W],
                        start=True,
                        stop=False,
                    )
                    nc.tensor.matmul(
                        out_sl,
                        xt_sb[:, 1, ms],
                        ytg_sb[:, 1, c0 : c0 + NS_W],
                        start=False,
                        stop=False,
                    )
                    nc.tensor.matmul(
                        out_sl,
                        ones_sb,
                        yaug_sb[:, c0 : c0 + NS_W],
                        start=False,
                        stop=True,
                    )
                e_t = e_pool.tile([P, NG_W], bf16)
                nc.scalar.activation(
                    out=e_t,
                    in_=psum,
                    func=mybir.ActivationFunctionType.Exp,
                    bias=xb_sb[:, mb : mb + 1],
                    scale=xb_sb[:, MB : MB + 1],
                    accum_out=st_sb[:, t : t + 1],
                )
                e_list.append(e_t)
                t += 1

        nc.vector.tensor_reduce(
            out=red_sb,
            in_=st_sb,
            axis=mybir.AxisListType.X,
            op=mybir.AluOpType.add,
        )
        nc.sync.dma_start(out=stats.ap(), in_=red_sb)

    if fix_waits:
        _strip_self_waits(nc, mybir)
        _rebalance_waits(nc, mybir)
    nc.finalize()
    return nc


def _rebalance_waits(nc, mybir, max_waits=1):
    """Split over-budget sync waits into single-wait same-engine drains.

    Any instruction with more than `max_waits` waits gets a chain of
    no-op InstDrain instructions inserted just before it on the same
    engine, each carrying one of the excess waits. Engine streams are
    in-order, so the drains gate the instruction exactly as the
    original multi-wait would, with no reordering of dependencies
    (unlike hoisting waits onto earlier instructions, which can
    deadlock when the hoist target gates the wait's producer).
    """
    for func in nc.m.functions:
        for block in func.blocks:
            changed = False
            new_insts = []
            for inst in list(block.instructions):
                si = inst.sync_info
                if si is not None and len(si.on_wait) > max_waits:
                    waits = list(si.on_wait)
                    keep = waits[:max_waits]
                    for j, w in enumerate(waits[max_waits:]):
                        d = mybir.InstDrain(
                            name=f"{inst.name}-wsplit{j}",
                            ins=[],
                            outs=[],
                            bass_is_fusable=False,
                        )
                        d.engine = inst.engine
                        d.sync_info = mybir.SyncInfo(
                            on_wait=[w], on_update=[]
                        )
                        new_insts.append(d)
                        changed = True
                    inst.sync_info = mybir.SyncInfo(
                        on_wait=keep, on_update=si.on_update
                    )
                new_insts.append(inst)
            if changed:
                try:
                    block.instructions = new_insts
                except (AttributeError, TypeError):
                    block.instructions.clear()
                    block.instructions.extend(new_insts)


def _strip_self_waits(nc, mybir):
    """Drop same-engine semaphore waits (PE waiting on PE, etc).

    Engine queues execute in order, so a wait on the instruction's own
    engine semaphore is redundant at runtime; Tile emits them
    conservatively for slot-recycle WAW hazards, but this walrus build
    only allows one sync wait per instruction. DMA-queue semaphores are
    never touched.
    """
    compute = ("PE", "Activation", "DVE", "Pool", "SP")
    for inst in nc.inst_map.values():
        si = inst.sync_info
        if si is None or not si.on_wait:
            continue
        prefix = str(inst.engine).split(".")[-1] + "_"
        if not prefix.startswith(compute):
            continue
        kept = [w for w in si.on_wait if not w.ant_name.startswith(prefix)]
        if len(kept) != len(si.on_wait):
            inst.sync_info = mybir.SyncInfo(on_wait=kept, on_update=si.on_update)


def check_waits(nc, max_waits=1):
    """Count instructions exceeding the per-instruction sync-wait budget."""
    bad = []
    for name, inst in nc.inst_map.items():
        si = inst.sync_info
        if si is not None and len(si.on_wait) > max_waits:
            bad.append(
                (
                    name,
                    type(inst).__name__,
                    [(w.ant_name, w.wait_value) for w in si.on_wait],
                )
            )
    return bad


_f8_lut = None


def _to_f8(dst_u8, src_f32, s):
    """fp32 -> fp8 e4m3 via fp16 + 64K-entry LUT (2.4x numpy's direct cast).

    The double rounding (fp32->fp16->fp8) differs from direct rounding by
    at most 1 fp8 ulp on ties — irrelevant at fp8's 2^-4 relative error.
    """
    global _f8_lut
    if _f8_lut is None:
        with np.errstate(invalid="ignore", over="ignore"):
            all16 = np.arange(65536, dtype=np.uint16).view(np.float16)
            _f8_lut = (
                all16.astype(np.float32)
                .astype(ml_dtypes.float8_e4m3)
                .view(np.uint8)
            )
    if s == 1.0:
        h = src_f32.astype(np.float16)
    else:
        h = (src_f32 * s).astype(np.float16)
    dst_u8[...] = _f8_lut[h.view(np.uint16)]


def _pick_scale(x, y):
    """Choose the fp8 range scale from a subsample (full stats only when
    the input is outside fp8's comfortable range and scaling is needed)."""
    xs = x.reshape(-1)[:: N * K // 16384]
    ys = y.reshape(-1)[:: M * K // 16384]
    rms2 = (np.square(xs).mean() + np.square(ys).mean()) / 2.0
    if 0.25 <= rms2 <= 64.0:
        return np.float32(1.0)
    amax = float(max(x.max(), -x.min(), y.max(), -y.min(), 1e-30))
    return np.float32(min(16.0 / np.sqrt(max(rms2, 1e-30)), 200.0 / amax))


def _prep_staged(x, y, sh):
    """Fast-path prep: cast both feature shard sets into ONE packed array
    and start its (async) device transfer, overlapping the remaining host
    work. A single put amortizes the ~30-40ms fixed cost each sharded
    device_put carries on the tunneled link."""
    import jax

    bf16 = ml_dtypes.bfloat16
    f8 = ml_dtypes.float8_e4m3
    x = np.asarray(x, dtype=np.float32)
    y = np.asarray(y, dtype=np.float32)
    xr = x.reshape(NCORES, MPC, K)
    yr = y.reshape(NCORES, MPC, K)

    s = _pick_scale(x, y)
    inv_s2 = np.float32(1.0) / (s * s)

    xy_g = np.empty((NCORES, 2 * K, MPC), f8)
    xy_u8 = xy_g.view(np.uint8)
    for c in range(NCORES):
        _to_f8(xy_u8[c, :K], xr[c].T, s)
        _to_f8(xy_u8[c, K:], yr[c].T, s)
    dxy = jax.device_put(xy_g.reshape(NCORES * 2 * K, MPC), sh)

    x2 = np.einsum("ij,ij->i", x, x).reshape(NCORES, MPC)
    y2 = np.einsum("ij,ij->i", y, y).reshape(NCORES, MPC)
    cv = (-0.5 * (s * s)) * y2
    c1 = cv.astype(bf16)
    c2 = (cv - c1.astype(np.float32)).astype(bf16)
    ya_g = np.empty((NCORES, 2, MPC), bf16)
    ya_g[:, 0] = c1
    ya_g[:, 1] = c2
    xb_g = np.empty((NCORES, P, MB + 1), np.float32)
    xb_g[:, :, :MB] = (-0.5 * x2).reshape(NCORES, MB, P).transpose(0, 2, 1)
    xb_g[:, :, MB] = inv_s2
    return {
        "xy": dxy,
        "ya": ya_g.reshape(NCORES * 2, MPC),
        "xb": xb_g.reshape(NCORES * P, MB + 1),
    }


def _prep(x, y):
    """Host-side layout: scaled fp8 feature shards + tiny O(N*K) row stats.

    Features ship as s*x, s*y in fp8 e4m3 (s sized so the rms lands at 16,
    well inside fp8's normal range); the fp32-accurate psum is rescaled on
    ACT via scale=1/s^2 shipped in xb's last column. The y-column term
    ships as two error-compensated bf16 rows computed from the SCALED y,
    so scale*(s^2 x.y + c1 + c2) = x.y - 0.5||y||^2 to ~fp32 accuracy.

    When the input rms is already inside fp8's comfortable range, s=1 and
    the scale multiply is skipped (only the <1% of elements below fp8's
    normal range lose precision, a negligible share of any dot product).
    """
    bf16 = ml_dtypes.bfloat16
    f8 = ml_dtypes.float8_e4m3
    x = np.asarray(x, dtype=np.float32)
    y = np.asarray(y, dtype=np.float32)

    xr = x.reshape(NCORES, MPC, K)
    yr = y.reshape(NCORES, MPC, K)
    x2 = np.einsum("ij,ij->i", x, x).reshape(NCORES, MPC)
    y2 = np.einsum("ij,ij->i", y, y).reshape(NCORES, MPC)

    rms2 = (x2.mean() + y2.mean()) / (2.0 * K)
    if 0.25 <= rms2 <= 64.0:
        s = np.float32(1.0)
    else:
        amax = float(max(x.max(), -x.min(), y.max(), -y.min(), 1e-30))
        s = np.float32(min(16.0 / np.sqrt(max(rms2, 1e-30)), 200.0 / amax))
    inv_s2 = np.float32(1.0) / (s * s)

    xy_g = np.empty((NCORES, 2 * K, MPC), f8)
    xy_u8 = xy_g.view(np.uint8)
    for c in range(NCORES):
        _to_f8(xy_u8[c, :K], xr[c].T, s)
        _to_f8(xy_u8[c, K:], yr[c].T, s)

    cv = (-0.5 * (s * s)) * y2                            # [NCORES, MPC] f32
    c1 = cv.astype(bf16)
    c2 = (cv - c1.astype(np.float32)).astype(bf16)
    ya_g = np.empty((NCORES, 2, MPC), bf16)
    ya_g[:, 0] = c1
    ya_g[:, 1] = c2
    xb_g = np.empty((NCORES, P, MB + 1), np.float32)
    xb_g[:, :, :MB] = (-0.5 * x2).reshape(NCORES, MB, P).transpose(0, 2, 1)
    xb_g[:, :, MB] = inv_s2
    return {"xy": xy_g, "ya": ya_g, "xb": xb_g}


def _build_fast_runner(nc):
    """Process-cached jit(shard_map) over the same bass2jax primitives
    run_bass_kernel_spmd uses, so repeat calls skip retrace + recompile."""
    import jax
    from jax.sharding import Mesh, PartitionSpec
    from jax.experimental.shard_map import shard_map
    import concourse.mybir as mybir
    from concourse.bass2jax import (
        _bass_exec_p,
        partition_id_tensor,
        install_neuronx_cc_hook,
    )

    install_neuronx_cc_hook()

    in_names, out_names, out_avals = [], [], []
    partition_name = (
        nc.partition_id_tensor.name if nc.partition_id_tensor else None
    )
    for alloc in nc.m.functions[0].allocations:
        if not isinstance(alloc, mybir.MemoryLocationSet):
            continue
        name = alloc.memorylocations[0].name
        if alloc.kind == "ExternalInput":
            if name != partition_name:
                in_names.append(name)
        elif alloc.kind == "ExternalOutput":
            out_names.append(name)
            shape = tuple(alloc.tensor_shape)
            dtype = mybir.dt.np(alloc.dtype)
            out_avals.append(jax.core.ShapedArray(shape, dtype))
    n_params = len(in_names)
    n_outs = len(out_avals)
    all_in_names = in_names + out_names + (
        [partition_name] if partition_name else []
    )
    donate = tuple(range(n_params, n_params + n_outs))

    def _body(*args):
        operands = list(args)
        if partition_name is not None:
            operands.append(partition_id_tensor())
        return tuple(
            _bass_exec_p.bind(
                *operands,
                out_avals=tuple(out_avals),
                in_names=tuple(all_in_names),
                out_names=tuple(out_names),
                lowering_input_output_aliases=(),
                sim_require_finite=True,
                sim_require_nnan=True,
                nc=nc,
            )
        )

    devices = jax.devices()[:NCORES]
    mesh = Mesh(np.asarray(devices), ("core",))
    sharded = jax.jit(
        shard_map(
            _body,
            mesh=mesh,
            in_specs=(PartitionSpec("core"),) * (n_params + n_outs),
            out_specs=(PartitionSpec("core"),) * n_outs,
            check_rep=False,
        ),
        donate_argnums=donate,
        keep_unused=True,
    )
    from jax.sharding import NamedSharding

    row_sharded = NamedSharding(mesh, PartitionSpec("core"))
    return {
        "sharded": sharded,
        "in_names": in_names,
        "out_names": out_names,
        "out_avals": out_avals,
        "row_sharded": row_sharded,
        "devices": devices,
    }


def kernel(x: np.ndarray, y: np.ndarray) -> np.ndarray:
    from concourse.bass_utils import run_bass_kernel_spmd

    if "nc" not in _cached:
        _cached["nc"] = _build()
    nc = _cached["nc"]

    fast = _cached.get("fast")
    if fast is not None:
        args = _prep_staged(x, y, fast["row_sharded"])
        concat_in = [args[n] for n in fast["in_names"]]
        concat_zeros = [
            np.zeros((NCORES * a.shape[0], *a.shape[1:]), a.dtype)
            for a in fast["out_avals"]
        ]
        outs = fast["sharded"](*concat_in, *concat_zeros)
        stats = np.asarray(outs[fast["out_names"].index("stats")])
    else:
        # First call (or fast-runner build failure): compile + run through
        # bass_utils.run_bass_kernel_spmd.
        in_by_name = _prep(x, y)
        in_maps = [
            {k: v[c] for k, v in in_by_name.items()} for c in range(NCORES)
        ]
        global _last_in_maps
        _last_in_maps = in_maps
        res = run_bass_kernel_spmd(
            nc, in_maps, core_ids=list(range(NCORES))
        )
        stats = np.stack([r["stats"] for r in res.results])
        if "fast" not in _cached:
            try:
                _cached["fast"] = _build_fast_runner(nc)
            except Exception:
                _cached["fast"] = None

    total = stats.astype(np.float64).sum()
    return np.float32(total / (float(N) * float(M)))


# revision 33
# speedup vs baseline: 2.0696x; 2.0696x over previous
"""Gaussian RBF kernel-mean loss on 8 Trainium2 NeuronCores.

Computes mean(exp(-||x_i - y_j||^2 / 2)) over all (i, j) pairs for
x, y of shape [8192, 256] fp32.

Math used on device (per core, rows of x sharded 1024/core):
    exp(-d2/2) = exp(x.y - 0.5||x||^2 - 0.5||y||^2)
Features ship as s*x, s*y in fp8 e4m3 (host picks s; s=1 fast path for
unit-scale inputs). The column term -0.5||s y_n||^2 rides in the
contraction as two error-compensated bf16 rows (c1 = bf16(v),
c2 = bf16(v - c1)) multiplied by constant-1 rows on the x side:
    psum = s^2 x.y + c1 + c2       # PE, fp32 accumulate, mixed-dtype group
    E    = exp(psum/s^2 + bias_m)  # ACT: scale=1/s^2 (xb last col),
                                   #      bias = per-partition -0.5||x_m||^2
    stats[:, t] = sum_n E          # ACT accum_out, fp32
    out[p] = sum_t stats[p, t]     # DVE tensor_reduce
The host adds the 8 * 128 partials and divides by N*M.

Distribution: a call's wall-clock is dominated by shipping bytes over
the tunneled PJRT link (the device kernel itself is ~100us), so each
core receives only its own x shard and y shard (~0.5MB fp8 per core);
the full y^T is assembled ON DEVICE with a DRAM AllGather across the 8
cores. Total host->device traffic ~4.4MB vs 54.6MB for the replicated
bf16 layout of the original version.

Execution: the first call goes through bass_utils.run_bass_kernel_spmd
(which compiles the NEFF). Subsequent calls reuse a process-cached
jax.jit(shard_map) wrapper built on the same bass2jax primitives, which
skips the per-call retrace + walrus recompile that run_bass_kernel_spmd
pays (it constructs a fresh jit closure per invocation).

Toolchain constraint: this walrus build accepts at most ONE sync wait
per compute/DMA instruction. The kernel keeps a strict PE -> ACT
pipeline; slot-recycle and DMA-arrival waits are absorbed by tiny
same-engine "observer" ops (LDWEIGHTS on PE, a scalar warmup on ACT),
and any instruction still over budget gets its excess waits split into
single-wait same-engine drains (_strip_self_waits/_rebalance_waits).
"""

import numpy as np
import ml_dtypes

N = 8192          # rows of x
M = 8192          # rows of y
K = 256           # feature dim
NCORES = 8
MPC = N // NCORES        # 1024 rows of x (and y) per core
P = 128                  # partitions
KO = K // P              # 2 k-chunks
KA = K + 2               # y shard rows incl c1, c2 compensation rows
MB = MPC // P            # 8 m-blocks per core
NG_W = 2048              # columns per psum tile (4 banks)
NG = M // NG_W           # 4 n-groups
NS_W = 512               # matmul free width (1 psum bank)
NS = NG_W // NS_W        # 4
NTILES = MB * NG         # 32 output tiles per core

_cached = {}
_last_in_maps = None


def _build(fix_waits=True):
    import concourse.bass as bass
    import concourse.tile as tile
    import concourse.mybir as mybir
    from contextlib import ExitStack

    fp32 = mybir.dt.float32
    bf16 = mybir.dt.bfloat16
    f8 = mybir.dt.float8e4

    nc = bass.Bass(trn_type="TRN2")
    # xy packs this core's x.T shard (rows 0..K-1) and y.T shard
    # (rows K..2K-1) into ONE input: a single device_put per call ships
    # both, halving the ~30-40ms fixed cost each put carries on the
    # tunneled PJRT link.
    xy = nc.dram_tensor("xy", [2 * K, MPC], f8, kind="ExternalInput")
    ya = nc.dram_tensor("ya", [2, MPC], bf16, kind="ExternalInput")
    xb = nc.dram_tensor("xb", [P, MB + 1], fp32, kind="ExternalInput")
    stats = nc.dram_tensor("stats", [P, 1], fp32, kind="ExternalOutput")

    with ExitStack() as ctx:
        tc = ctx.enter_context(tile.TileContext(nc))
        singles = ctx.enter_context(tc.tile_pool(name="singles", bufs=1))
        dram = ctx.enter_context(tc.tile_pool(name="dram", bufs=1, space="DRAM"))
        psum_pool = ctx.enter_context(
            tc.tile_pool(name="psum", bufs=2, space="PSUM")
        )
        e_pool = ctx.enter_context(tc.tile_pool(name="e", bufs=4))

        in_f = dram.tile([K, MPC], f8)
        in_a = dram.tile([2, MPC], bf16)
        ytg = dram.tile([NCORES, K, MPC], f8)
        ytga = dram.tile([NCORES, 2, MPC], bf16)

        xt_sb = singles.tile([P, KO, MPC], f8)
        ytg_sb = singles.tile([P, KO, M], f8)
        yaug_sb = singles.tile([2, M], bf16)
        ones_sb = singles.tile([2, P], bf16)
        xb_sb = singles.tile([P, MB + 1], fp32)
        st_sb = singles.tile([P, NTILES], fp32)
        red_sb = singles.tile([P, 1], fp32)
        warm = singles.tile([P, 1], fp32)

        # y shard -> DRAM bounce -> AllGather to full y^T (+aug rows)
        nc.gpsimd.dma_start(out=in_f, in_=xy[K : 2 * K, :])
        nc.gpsimd.dma_start(out=in_a, in_=ya.ap())
        nc.gpsimd.collective_compute(
            "AllGather",
            mybir.AluOpType.bypass,
            replica_groups=[list(range(NCORES))],
            ins=[in_f.opt()],
            outs=[ytg.opt()],
        )
        nc.gpsimd.collective_compute(
            "AllGather",
            mybir.AluOpType.bypass,
            replica_groups=[list(range(NCORES))],
            ins=[in_a.opt()],
            outs=[ytga.opt()],
        )

        for ko in range(KO):
            nc.sync.dma_start(
                out=xt_sb[:, ko], in_=xy[ko * P : (ko + 1) * P, :]
            )
        nc.sync.dma_start(out=xb_sb, in_=xb.ap())
        nc.vector.memset(ones_sb, 1.0)
        # PE observer for the xt DMA queue (no PSUM write -> no bank WAW)
        nc.tensor.ldweights(weights=xt_sb[:, 0, 0:P])
        # ACT warmup: loads the exp table set AND observes the xb DMA queue,
        # so no later Exp carries the table-load's extra sync wait.
        nc.scalar.activation(
            out=warm, in_=xb_sb[:, 0:1], func=mybir.ActivationFunctionType.Exp
        )
        # gathered y columns: per source core j, feature chunks + aug rows
        for j in range(NCORES):
            cs = slice(j * MPC, (j + 1) * MPC)
            for ko in range(KO):
                nc.sync.dma_start(
                    out=ytg_sb[:, ko, cs],
                    in_=ytg[j, ko * P : (ko + 1) * P, :],
                )
            nc.sync.dma_start(out=yaug_sb[:, cs], in_=ytga[j])

        e_list = []
        t = 0
        for mb in range(MB):
            ms = slice(mb * P, (mb + 1) * P)
            for ng in range(NG):
                if mb == 0:
                    # PE observer: absorb the ytg shard DMA-arrival waits
                    j0 = ng * (NG_W // MPC)
                    c0 = j0 * MPC
                    nc.tensor.ldweights(weights=ytg_sb[:, 0, c0 : c0 + P])
                if t >= 2:
                    # PE observer: absorb the psum-slot-recycle wait
                    # (ACT finished exp of tile t-2).
                    nc.tensor.ldweights(weights=e_list[t - 2][:, 0:P])
                psum = psum_pool.tile([P, NG_W], fp32)
                for ns in range(NS):
                    c0 = ng * NG_W + ns * NS_W
                    out_sl = psum[:, ns * NS_W : (ns + 1) * NS_W]
                    nc.tensor.matmul(
                        out_sl,
                        xt_sb[:, 0, ms],
                        ytg_sb[:, 0, c0 : c0 + NS_W],
                        start=True,
                        stop=False,
                    )
                    nc.tensor.matmul(
                        out_sl,
                        xt_sb[:, 1, ms],
                        ytg_sb[:, 1, c0 : c0 + NS_W],
                        start=False,
                        stop=False,
                    )
                    nc.tensor.matmul(
                        out_sl,
                        ones_sb,
                        yaug_sb[:, c0 : c0 + NS_W],
                        start=False,
                        stop=True,
                    )
                e_t = e_pool.tile([P, NG_W], bf16)
                nc.scalar.activation(
                    out=e_t,
                    in_=psum,
                    func=mybir.ActivationFunctionType.Exp,
                    bias=xb_sb[:, mb : mb + 1],
                    scale=xb_sb[:, MB : MB + 1],
                    accum_out=st_sb[:, t : t + 1],
                )
                e_list.append(e_t)
                t += 1

        nc.vector.tensor_reduce(
            out=red_sb,
            in_=st_sb,
            axis=mybir.AxisListType.X,
            op=mybir.AluOpType.add,
        )
        nc.sync.dma_start(out=stats.ap(), in_=red_sb)

    if fix_waits:
        _strip_self_waits(nc, mybir)
        _rebalance_waits(nc, mybir)
    nc.finalize()
    return nc


def _rebalance_waits(nc, mybir, max_waits=1):
    """Split over-budget sync waits into single-wait same-engine drains.

    Any instruction with more than `max_waits` waits gets a chain of
    no-op InstDrain instructions inserted just before it on the same
    engine, each carrying one of the excess waits. Engine streams are
    in-order, so the drains gate the instruction exactly as the
    original multi-wait would, with no reordering of dependencies
    (unlike hoisting waits onto earlier instructions, which can
    deadlock when the hoist target gates the wait's producer).
    """
    for func in nc.m.functions:
        for block in func.blocks:
            changed = False
            new_insts = []
            for inst in list(block.instructions):
                si = inst.sync_info
                if si is not None and len(si.on_wait) > max_waits:
                    waits = list(si.on_wait)
                    keep = waits[:max_waits]
                    for j, w in enumerate(waits[max_waits:]):
                        d = mybir.InstDrain(
                            name=f"{inst.name}-wsplit{j}",
                            ins=[],
                            outs=[],
                            bass_is_fusable=False,
                        )
                        d.engine = inst.engine
                        d.sync_info = mybir.SyncInfo(
                            on_wait=[w], on_update=[]
                        )
                        new_insts.append(d)
                        changed = True
                    inst.sync_info = mybir.SyncInfo(
                        on_wait=keep, on_update=si.on_update
                    )
                new_insts.append(inst)
            if changed:
                try:
                    block.instructions = new_insts
                except (AttributeError, TypeError):
                    block.instructions.clear()
                    block.instructions.extend(new_insts)


def _strip_self_waits(nc, mybir):
    """Drop same-engine semaphore waits (PE waiting on PE, etc).

    Engine queues execute in order, so a wait on the instruction's own
    engine semaphore is redundant at runtime; Tile emits them
    conservatively for slot-recycle WAW hazards, but this walrus build
    only allows one sync wait per instruction. DMA-queue semaphores are
    never touched.
    """
    compute = ("PE", "Activation", "DVE", "Pool", "SP")
    for inst in nc.inst_map.values():
        si = inst.sync_info
        if si is None or not si.on_wait:
            continue
        prefix = str(inst.engine).split(".")[-1] + "_"
        if not prefix.startswith(compute):
            continue
        kept = [w for w in si.on_wait if not w.ant_name.startswith(prefix)]
        if len(kept) != len(si.on_wait):
            inst.sync_info = mybir.SyncInfo(on_wait=kept, on_update=si.on_update)


def check_waits(nc, max_waits=1):
    """Count instructions exceeding the per-instruction sync-wait budget."""
    bad = []
    for name, inst in nc.inst_map.items():
        si = inst.sync_info
        if si is not None and len(si.on_wait) > max_waits:
            bad.append(
                (
                    name,
                    type(inst).__name__,
                    [(w.ant_name, w.wait_value) for w in si.on_wait],
                )
            )
    return bad


_f8_lut = None


def _to_f8(dst_u8, src_f32, s):
    """fp32 -> transposed fp8 e4m3 via fp16 + 64K-entry LUT.

    2.4x numpy's direct cast, and all heavy steps (f16 astype, LUT
    gather) run on CONTIGUOUS data — the layout transpose happens last,
    on the 1-byte output, where it moves the fewest bytes. Writes
    src.T's fp8 bytes into dst_u8. The double rounding
    (fp32->fp16->fp8) differs from direct rounding by at most 1 fp8 ulp
    on ties — irrelevant at fp8's 2^-4 relative error.
    """
    global _f8_lut
    if _f8_lut is None:
        with np.errstate(invalid="ignore", over="ignore"):
            all16 = np.arange(65536, dtype=np.uint16).view(np.float16)
            _f8_lut = (
                all16.astype(np.float32)
                .astype(ml_dtypes.float8_e4m3)
                .view(np.uint8)
            )
    if s == 1.0:
        h = src_f32.astype(np.float16)
    else:
        h = (src_f32 * s).astype(np.float16)
    dst_u8[...] = _f8_lut[h.view(np.uint16)].T


def _pick_scale(x, y):
    """Choose the fp8 range scale from a subsample (full stats only when
    the input is outside fp8's comfortable range and scaling is needed)."""
    xs = x.reshape(-1)[:: N * K // 16384]
    ys = y.reshape(-1)[:: M * K // 16384]
    rms2 = (np.square(xs).mean() + np.square(ys).mean()) / 2.0
    if 0.25 <= rms2 <= 64.0:
        return np.float32(1.0)
    amax = float(max(x.max(), -x.min(), y.max(), -y.min(), 1e-30))
    return np.float32(min(16.0 / np.sqrt(max(rms2, 1e-30)), 200.0 / amax))


def _prep_staged(x, y, sh):
    """Fast-path prep: cast both feature shard sets into ONE packed array
    and start its (async) device transfer, overlapping the remaining host
    work. A single put amortizes the ~30-40ms fixed cost each sharded
    device_put carries on the tunneled link."""
    import jax

    bf16 = ml_dtypes.bfloat16
    f8 = ml_dtypes.float8_e4m3
    x = np.asarray(x, dtype=np.float32)
    y = np.asarray(y, dtype=np.float32)
    xr = x.reshape(NCORES, MPC, K)
    yr = y.reshape(NCORES, MPC, K)

    s = _pick_scale(x, y)
    inv_s2 = np.float32(1.0) / (s * s)

    xy_g = np.empty((NCORES, 2 * K, MPC), f8)
    xy_u8 = xy_g.view(np.uint8)
    for c in range(NCORES):
        _to_f8(xy_u8[c, :K], xr[c], s)
        _to_f8(xy_u8[c, K:], yr[c], s)
    dxy = jax.device_put(xy_g.reshape(NCORES * 2 * K, MPC), sh)

    x2 = np.einsum("ij,ij->i", x, x).reshape(NCORES, MPC)
    y2 = np.einsum("ij,ij->i", y, y).reshape(NCORES, MPC)
    cv = (-0.5 * (s * s)) * y2
    c1 = cv.astype(bf16)
    c2 = (cv - c1.astype(np.float32)).astype(bf16)
    ya_g = np.empty((NCORES, 2, MPC), bf16)
    ya_g[:, 0] = c1
    ya_g[:, 1] = c2
    xb_g = np.empty((NCORES, P, MB + 1), np.float32)
    xb_g[:, :, :MB] = (-0.5 * x2).reshape(NCORES, MB, P).transpose(0, 2, 1)
    xb_g[:, :, MB] = inv_s2
    return {
        "xy": dxy,
        "ya": ya_g.reshape(NCORES * 2, MPC),
        "xb": xb_g.reshape(NCORES * P, MB + 1),
    }


def _prep(x, y):
    """Host-side layout: scaled fp8 feature shards + tiny O(N*K) row stats.

    Features ship as s*x, s*y in fp8 e4m3 (s sized so the rms lands at 16,
    well inside fp8's normal range); the fp32-accurate psum is rescaled on
    ACT via scale=1/s^2 shipped in xb's last column. The y-column term
    ships as two error-compensated bf16 rows computed from the SCALED y,
    so scale*(s^2 x.y + c1 + c2) = x.y - 0.5||y||^2 to ~fp32 accuracy.

    When the input rms is already inside fp8's comfortable range, s=1 and
    the scale multiply is skipped (only the <1% of elements below fp8's
    normal range lose precision, a negligible share of any dot product).
    """
    bf16 = ml_dtypes.bfloat16
    f8 = ml_dtypes.float8_e4m3
    x = np.asarray(x, dtype=np.float32)
    y = np.asarray(y, dtype=np.float32)

    xr = x.reshape(NCORES, MPC, K)
    yr = y.reshape(NCORES, MPC, K)
    x2 = np.einsum("ij,ij->i", x, x).reshape(NCORES, MPC)
    y2 = np.einsum("ij,ij->i", y, y).reshape(NCORES, MPC)

    rms2 = (x2.mean() + y2.mean()) / (2.0 * K)
    if 0.25 <= rms2 <= 64.0:
        s = np.float32(1.0)
    else:
        amax = float(max(x.max(), -x.min(), y.max(), -y.min(), 1e-30))
        s = np.float32(min(16.0 / np.sqrt(max(rms2, 1e-30)), 200.0 / amax))
    inv_s2 = np.float32(1.0) / (s * s)

    xy_g = np.empty((NCORES, 2 * K, MPC), f8)
    xy_u8 = xy_g.view(np.uint8)
    for c in range(NCORES):
        _to_f8(xy_u8[c, :K], xr[c], s)
        _to_f8(xy_u8[c, K:], yr[c], s)

    cv = (-0.5 * (s * s)) * y2                            # [NCORES, MPC] f32
    c1 = cv.astype(bf16)
    c2 = (cv - c1.astype(np.float32)).astype(bf16)
    ya_g = np.empty((NCORES, 2, MPC), bf16)
    ya_g[:, 0] = c1
    ya_g[:, 1] = c2
    xb_g = np.empty((NCORES, P, MB + 1), np.float32)
    xb_g[:, :, :MB] = (-0.5 * x2).reshape(NCORES, MB, P).transpose(0, 2, 1)
    xb_g[:, :, MB] = inv_s2
    return {"xy": xy_g, "ya": ya_g, "xb": xb_g}


def _build_fast_runner(nc):
    """Process-cached jit(shard_map) over the same bass2jax primitives
    run_bass_kernel_spmd uses, so repeat calls skip retrace + recompile."""
    import jax
    from jax.sharding import Mesh, PartitionSpec
    from jax.experimental.shard_map import shard_map
    import concourse.mybir as mybir
    from concourse.bass2jax import (
        _bass_exec_p,
        partition_id_tensor,
        install_neuronx_cc_hook,
    )

    install_neuronx_cc_hook()

    in_names, out_names, out_avals = [], [], []
    partition_name = (
        nc.partition_id_tensor.name if nc.partition_id_tensor else None
    )
    for alloc in nc.m.functions[0].allocations:
        if not isinstance(alloc, mybir.MemoryLocationSet):
            continue
        name = alloc.memorylocations[0].name
        if alloc.kind == "ExternalInput":
            if name != partition_name:
                in_names.append(name)
        elif alloc.kind == "ExternalOutput":
            out_names.append(name)
            shape = tuple(alloc.tensor_shape)
            dtype = mybir.dt.np(alloc.dtype)
            out_avals.append(jax.core.ShapedArray(shape, dtype))
    n_params = len(in_names)
    n_outs = len(out_avals)
    all_in_names = in_names + out_names + (
        [partition_name] if partition_name else []
    )
    donate = tuple(range(n_params, n_params + n_outs))

    def _body(*args):
        operands = list(args)
        if partition_name is not None:
            operands.append(partition_id_tensor())
        return tuple(
            _bass_exec_p.bind(
                *operands,
                out_avals=tuple(out_avals),
                in_names=tuple(all_in_names),
                out_names=tuple(out_names),
                lowering_input_output_aliases=(),
                sim_require_finite=True,
                sim_require_nnan=True,
                nc=nc,
            )
        )

    devices = jax.devices()[:NCORES]
    mesh = Mesh(np.asarray(devices), ("core",))
    sharded = jax.jit(
        shard_map(
            _body,
            mesh=mesh,
            in_specs=(PartitionSpec("core"),) * (n_params + n_outs),
            out_specs=(PartitionSpec("core"),) * n_outs,
            check_rep=False,
        ),
        donate_argnums=donate,
        keep_unused=True,
    )
    from jax.sharding import NamedSharding

    row_sharded = NamedSharding(mesh, PartitionSpec("core"))
    return {
        "sharded": sharded,
        "in_names": in_names,
        "out_names": out_names,
        "out_avals": out_avals,
        "row_sharded": row_sharded,
        "devices": devices,
    }


def kernel(x: np.ndarray, y: np.ndarray) -> np.ndarray:
    from concourse.bass_utils import run_bass_kernel_spmd

    if "nc" not in _cached:
        _cached["nc"] = _build()
    nc = _cached["nc"]

    fast = _cached.get("fast")
    if fast is not None:
        args = _prep_staged(x, y, fast["row_sharded"])
        concat_in = [args[n] for n in fast["in_names"]]
        concat_zeros = [
            np.zeros((NCORES * a.shape[0], *a.shape[1:]), a.dtype)
            for a in fast["out_avals"]
        ]
        outs = fast["sharded"](*concat_in, *concat_zeros)
        stats = np.asarray(outs[fast["out_names"].index("stats")])
    else:
        # First call (or fast-runner build failure): compile + run through
        # bass_utils.run_bass_kernel_spmd.
        in_by_name = _prep(x, y)
        in_maps = [
            {k: v[c] for k, v in in_by_name.items()} for c in range(NCORES)
        ]
        global _last_in_maps
        _last_in_maps = in_maps
        res = run_bass_kernel_spmd(
            nc, in_maps, core_ids=list(range(NCORES))
        )
        stats = np.stack([r["stats"] for r in res.results])
        if "fast" not in _cached:
            try:
                _cached["fast"] = _build_fast_runner(nc)
            except Exception:
                _cached["fast"] = None

    total = stats.astype(np.float64).sum()
    return np.float32(total / (float(N) * float(M)))
